# revision 15
# baseline (speedup 1.0000x reference)
"""
Trainium2 Bass kernel for nn_ABSA_Lstm: masked LSTM over ragged sequences.

  reference:  x = emb[sent]; LSTM over T=128 steps with per-sequence length
              masking; out = h_final @ Wout.T + bout   -> [256, 3]

Strategy (8 NeuronCores, data parallel):
  - Shard batch B=256 -> 32 sequences per core. LSTM weights replicated.
  - Host does the embedding-table gather (pure data movement) and packs
    transposed/padded tile layouts; all model FLOPs run on device:
      phase 1: gates_x[b,t,:] = x[b,t,:] @ Wih_r.T + (b_ih+b_hh)   (big matmul)
      phase 2: 128 sequential LSTM cell steps (h.T is the matmul stationary)
      phase 3: out = h_cap @ Wout.T + bout
  - Ragged lengths: the recurrence runs unmasked; h is *captured* into Hf at
    t == len[b]-1 via a per-partition one-hot scalar (off the critical path).
    This is exact: for t >= len the reference state is frozen, so the captured
    h_{len-1} equals the reference h_T.

Gate order is permuted (i,f,g,o) -> (i,f,o,g) on the host so sigmoid applies
to one contiguous [.,900] slab and tanh to [.,300].
"""

import numpy as np
import ml_dtypes

import concourse.bass as bass
import concourse.tile as tile
from concourse import mybir
from concourse.bass_utils import run_bass_kernel_spmd

BF16 = ml_dtypes.bfloat16

# Model dims (hardcoded per spec nn_ABSA_Lstm_377957122440)
VOCAB, TVOCAB, D, H, C, B, T = 100000, 2000, 300, 300, 3, 256, 128
NCORES = 8
BL = B // NCORES          # 32 local batch
KT = 3                    # K tiles of 128 covering D(+1 bias row) / H
TC = T // 4               # 32 M-tiles of (4 t's x 32 b) in phase 1
NCH = 3                   # 1200 gate dims as 3 chunks of 400
CH = 400

_cache = {}


def _build_graph(legalize=True, debug=False, t_steps=T, reps=1,
                 trace_sim=False):
    nc = bass.Bass()
    f32 = mybir.dt.float32
    bf16 = mybir.dt.bfloat16

    # ---- DRAM I/O ----
    xT = nc.dram_tensor("xT", [128, TC, KT, 128], bf16, kind="ExternalInput")
    wihT = nc.dram_tensor("wihT", [128, KT, 1200], bf16, kind="ExternalInput")
    whhT = nc.dram_tensor("whhT", [128, KT, 1200], bf16, kind="ExternalInput")
    wot = nc.dram_tensor("wot", [128, KT, 8], bf16, kind="ExternalInput")
    boutb = nc.dram_tensor("boutb", [BL, C], f32, kind="ExternalInput")
    h0T = nc.dram_tensor("h0T", [128, KT, BL], bf16, kind="ExternalInput")
    c0 = nc.dram_tensor("c0", [BL, H], f32, kind="ExternalInput")
    mlast = nc.dram_tensor("mlast", [BL, T], f32, kind="ExternalInput")
    ident = nc.dram_tensor("ident", [32, 32], bf16, kind="ExternalInput")
    out = nc.dram_tensor("out", [BL, C], f32, kind="ExternalOutput")
    dbg = None
    if debug:
        dbg = {
            "dbg_gx": nc.dram_tensor("dbg_gx", [128, TC, 1200], bf16,
                                     kind="ExternalOutput"),
            "dbg_gates": nc.dram_tensor("dbg_gates", [BL, 1200], f32,
                                        kind="ExternalOutput"),
            "dbg_h": nc.dram_tensor("dbg_h", [BL, H], bf16,
                                    kind="ExternalOutput"),
            "dbg_hT": nc.dram_tensor("dbg_hT", [128, KT, BL], bf16,
                                     kind="ExternalOutput"),
        }

    with tile.TileContext(nc, trace_sim=trace_sim) as tc:
        for _ in range(reps):
            _body(nc, tc, xT, wihT, whhT, wot, boutb, h0T, c0, mlast, ident,
                  out, dbg, t_steps)
    if legalize:
        _legalize_single_wait(nc)
    return nc


def _legalize_single_wait(nc):
    """This walrus build accepts at most ONE sync wait per instruction.
    Hoist extra waits emitted by Tile onto standalone EventSemaphore
    instructions placed immediately before the offender on the same engine."""
    for fn in nc.m.functions:
        for b in fn.blocks:
            out = []
            for inst in b.instructions:
                si = getattr(inst, "sync_info", None)
                if si is not None and si.on_wait and len(si.on_wait) > 1:
                    for w in si.on_wait[:-1]:
                        out.append(mybir.InstEventSemaphore(
                            name=nc.get_next_instruction_name(),
                            engine=inst.engine,
                            ins=[], outs=[],
                            sync_info=mybir.SyncInfo(on_wait=[w], on_update=[]),
                        ))
                    si.on_wait = [si.on_wait[-1]]
                out.append(inst)
            b.instructions[:] = out


def TileCtx(nc):
    return tile.TileContext(nc)


def _body(nc, tc, xT, wihT, whhT, wot, boutb, h0T, c0, mlast, ident, out, dbg=None, t_steps=T):
    f32 = mybir.dt.float32
    bf16 = mybir.dt.bfloat16
    Sig = mybir.ActivationFunctionType.Sigmoid
    Tanh = mybir.ActivationFunctionType.Tanh
    MUL = mybir.AluOpType.mult
    ADD = mybir.AluOpType.add

    from contextlib import ExitStack

    with ExitStack() as ctx:
        singles = ctx.enter_context(tc.tile_pool(name="singles", bufs=1))

        # ---- resident tiles ----
        whh_sb = singles.tile([128, KT, 1200], bf16)
        nc.sync.dma_start(out=whh_sb, in_=whhT[:, :, :])
        mlast_sb = singles.tile([BL, T], f32)
        nc.sync.dma_start(out=mlast_sb, in_=mlast[:, :])
        ident_sb = singles.tile([32, 32], bf16)
        nc.sync.dma_start(out=ident_sb, in_=ident[:, :])
        boutb_sb = singles.tile([BL, C], f32)
        nc.sync.dma_start(out=boutb_sb, in_=boutb[:, :])

        gx_all = singles.tile([128, TC, 1200], bf16)
        xT_sb = singles.tile([128, TC, KT, 128], bf16)
        nc.sync.dma_start(out=xT_sb, in_=xT[:, :, :, :])

        c_sb = singles.tile([BL, H], f32)
        nc.sync.dma_start(out=c_sb, in_=c0[:, :])
        Hf = singles.tile([BL, H], f32)
        nc.vector.memset(Hf, 0.0)
        # ping-pong transposed hidden state [kt, 128, 32]
        hT = [singles.tile([128, KT, BL], bf16, name=f"hT{i}", tag=f"hT{i}")
              for i in range(2)]
        nc.vector.memset(hT[1], 0.0)
        nc.sync.dma_start(out=hT[0], in_=h0T[:, :, :])

        # ================= phase 1: gates_x = X @ WihT (+bias row) ==========
        with ExitStack() as p1:
            wih_sb = p1.enter_context(tc.tile_pool(name="wihp", bufs=1)).tile(
                [128, KT, 1200], bf16)
            nc.sync.dma_start(out=wih_sb, in_=wihT[:, :, :])
            ppool = p1.enter_context(tc.tile_pool(name="pp", bufs=2, space="PSUM"))
            for tcix in range(TC):
                ps = ppool.tile([128, NCH, 512], f32)
                for k in range(KT):
                    for n in range(NCH):
                        nc.tensor.matmul(
                            ps[:, n, 0:CH],
                            lhsT=xT_sb[:, tcix, k, :],
                            rhs=wih_sb[:, k, n * CH:(n + 1) * CH],
                            start=(k == 0), stop=(k == KT - 1),
                        )
                for n in range(NCH):
                    nc.any.tensor_copy(
                        gx_all[:, tcix, n * CH:(n + 1) * CH], ps[:, n, 0:CH])

        if dbg is not None:
            nc.sync.dma_start(out=dbg["dbg_gx"][:, :, :], in_=gx_all)

        # ================= phase 2: recurrence ==============================
        with ExitStack() as p2:
            pgpool = p2.enter_context(tc.tile_pool(name="pgp", bufs=1, space="PSUM"))
            ptpool = p2.enter_context(tc.tile_pool(name="ptp", bufs=3, space="PSUM"))
            ew = p2.enter_context(tc.tile_pool(name="ew", bufs=2))

            for t in range(t_steps):
                tcix, tt = t // 4, t % 4
                cur, nxt = hT[t % 2], hT[(t + 1) % 2]

                # gates_h = h @ Whh_r.T : PSUM [32, 3x400]
                pg = pgpool.tile([BL, NCH, 512], f32)
                for k in range(KT):
                    for n in range(NCH):
                        nc.tensor.matmul(
                            pg[:, n, 0:CH],
                            lhsT=cur[:, k, :],
                            rhs=whh_sb[:, k, n * CH:(n + 1) * CH],
                            start=(k == 0), stop=(k == KT - 1),
                        )

                # gates = gates_h + gates_x
                gates = ew.tile([BL, 1200], f32)
                nc.vector.tensor_tensor(
                    gates.rearrange("p (n c) -> p n c", n=NCH),
                    pg[:, :, 0:CH],
                    gx_all[32 * tt:32 * tt + 32, tcix, :].rearrange(
                        "p (n c) -> p n c", n=NCH), ADD)

                if dbg is not None and t == 0:
                    nc.sync.dma_start(out=dbg["dbg_gates"][:, :], in_=gates)

                # activations: [i f o] sigmoid, [g] tanh
                sig = ew.tile([BL, 900], f32)
                nc.scalar.activation(sig, gates[:, 0:900], Sig)
                gg = ew.tile([BL, H], f32)
                nc.scalar.activation(gg, gates[:, 900:1200], Tanh)

                # c = f*c + i*g
                t_ig = ew.tile([BL, H], f32)
                nc.vector.tensor_mul(t_ig, sig[:, 0:H], gg)
                t_fc = ew.tile([BL, H], f32)
                nc.vector.tensor_mul(t_fc, sig[:, H:2 * H], c_sb)
                nc.vector.tensor_add(c_sb, t_fc, t_ig)

                # h = o * tanh(c)
                th = ew.tile([BL, H], f32)
                nc.scalar.activation(th, c_sb, Tanh)
                h_bf = ew.tile([BL, H], bf16)
                nc.vector.tensor_mul(h_bf, sig[:, 2 * H:3 * H], th)

                if dbg is not None and t == 0:
                    nc.sync.dma_start(out=dbg["dbg_h"][:, :], in_=h_bf)

                # capture h at t == len-1:  Hf += (h - Hf) * mlast[:, t]
                dh = ew.tile([BL, H], f32)
                nc.vector.tensor_sub(dh, h_bf, Hf)
                nc.vector.scalar_tensor_tensor(
                    Hf, dh, mlast_sb[:, t:t + 1], Hf, MUL, ADD)

                # transpose h for next step's stationary
                for k in range(KT):
                    w = 128 if k < 2 else H - 256
                    pt = ptpool.tile([128, BL], bf16)
                    nc.tensor.transpose(
                        pt[0:w, :], h_bf[:, 128 * k:128 * k + w], ident_sb)
                    nc.any.tensor_copy(nxt[0:w, k, :], pt[0:w, :])
                if dbg is not None and t == 0:
                    nc.sync.dma_start(out=dbg["dbg_hT"][:, :, :], in_=nxt)

        # ================= phase 3: out = Hf @ WoutT + bout =================
        with ExitStack() as p3:
            fp = p3.enter_context(tc.tile_pool(name="fp", bufs=1))
            fps = p3.enter_context(tc.tile_pool(name="fps", bufs=4, space="PSUM"))
            wot_sb = fp.tile([128, KT, 8], bf16)
            nc.sync.dma_start(out=wot_sb, in_=wot[:, :, :])
            hf_bf = fp.tile([BL, H], bf16)
            nc.vector.tensor_copy(hf_bf, Hf)
            hfT = fp.tile([128, KT, BL], bf16)
            nc.vector.memset(hfT, 0.0)
            for k in range(KT):
                w = 128 if k < 2 else H - 256
                pt = fps.tile([128, BL], bf16)
                nc.tensor.transpose(pt[0:w, :], hf_bf[:, 128 * k:128 * k + w],
                                    ident_sb)
                nc.any.tensor_copy(hfT[0:w, k, :], pt[0:w, :])
            po = fps.tile([BL, 8], f32)
            for k in range(KT):
                nc.tensor.matmul(po[:, 0:C], lhsT=hfT[:, k, :], rhs=wot_sb[:, k, 0:C],
                                 start=(k == 0), stop=(k == KT - 1))
            o_sb = fp.tile([BL, C], f32)
            nc.vector.tensor_add(o_sb, po[:, 0:C], boutb_sb)
            nc.sync.dma_start(out=out[:, :], in_=o_sb)


def _prep_inputs(sent, target, lens, emb, Wih, Whh, b_ih, b_hh, h0, c0,
                 Wout, bout):
    """Host-side shard + layout packing (data movement / tiny reindexing only)."""
    # permute gate order (i,f,g,o) -> (i,f,o,g)
    perm = np.concatenate([np.arange(0, 300), np.arange(300, 600),
                           np.arange(900, 1200), np.arange(600, 900)])
    wih_r = Wih[perm].astype(np.float32)          # [1200, 300]
    whh_r = Whh[perm].astype(np.float32)
    bias_r = (b_ih + b_hh)[perm].astype(np.float32)

    # [p, kt, n] with row D==bias, zero padded
    wihT = np.zeros((128, KT, 1200), np.float32)
    whhT = np.zeros((128, KT, 1200), np.float32)
    for k in range(KT):
        lo, hi = 128 * k, min(128 * (k + 1), D)
        wihT[0:hi - lo, k, :] = wih_r[:, lo:hi].T
        whhT[0:hi - lo, k, :] = whh_r[:, lo:hi].T
    wihT[D - 256, 2, :] = bias_r                   # ones-row partner
    wot = np.zeros((128, KT, 8), np.float32)
    for k in range(KT):
        lo, hi = 128 * k, min(128 * (k + 1), H)
        wot[0:hi - lo, k, 0:C] = Wout[:, lo:hi].T

    ident = np.eye(32, dtype=np.float32)

    in_maps = []
    for ci in range(NCORES):
        sl = slice(ci * BL, (ci + 1) * BL)
        x = emb[sent[sl]].astype(np.float32)       # [32, 128, 300] gather
        xT = np.zeros((128, TC, KT, 128), np.float32)
        # lhsT layout: xT[p=dk, tc, kt, m=32tt+b] = x[b, 4tc+tt, 128kt+dk]
        xr = x.transpose(1, 0, 2).reshape(TC, 4, BL, D)   # [tc, tt, b, d]
        xr = xr.reshape(TC, 128, D)                       # [tc, m, d]
        for k in range(KT):
            lo, hi = 128 * k, min(128 * (k + 1), D)
            xT[0:hi - lo, :, k, :] = xr[:, :, lo:hi].transpose(2, 0, 1)
        xT[D - 256, :, 2, :] = 1.0                 # bias ones-row

        h0T = np.zeros((128, KT, BL), np.float32)
        for k in range(KT):
            lo, hi = 128 * k, min(128 * (k + 1), H)
            h0T[0:hi - lo, k, :] = h0[sl, lo:hi].T

        lloc = lens[sl].astype(np.int64)
        mlast = np.zeros((BL, T), np.float32)
        mlast[np.arange(BL), np.clip(lloc - 1, 0, T - 1)] = 1.0

        in_maps.append({
            "xT": xT.astype(BF16),
            "wihT": wihT.astype(BF16),
            "whhT": whhT.astype(BF16),
            "wot": wot.astype(BF16),
            "boutb": np.tile(bout.astype(np.float32), (BL, 1)),
            "h0T": h0T.astype(BF16),
            "c0": c0[sl].astype(np.float32),
            "mlast": mlast,
            "ident": ident.astype(BF16),
        })
    return in_maps


def kernel(**inputs):
    if "nc" not in _cache:
        _cache["nc"] = _build_graph()
    nc = _cache["nc"]
    in_maps = _prep_inputs(**inputs)
    res = run_bass_kernel_spmd(nc, in_maps, core_ids=list(range(NCORES)))
    outs = [res.results[i]["out"] for i in range(NCORES)]
    return np.concatenate(outs, axis=0).astype(np.float32)


# revision 28
# speedup vs baseline: 521.9372x; 521.9372x over previous
"""
Trainium2 Bass kernel for nn_ABSA_Lstm: masked LSTM over ragged sequences.

  reference:  x = emb[sent]; LSTM over T=128 steps with per-sequence length
              masking; out = h_final @ Wout.T + bout   -> [256, 3]

Strategy (8 NeuronCores, data parallel):
  - Shard batch B=256 -> 32 sequences per core. LSTM weights replicated.
  - Host does the embedding-table gather (pure data movement) and packs
    transposed/padded tile layouts; all model FLOPs run on device:
      phase 1: gates_x[b,t,:] = x[b,t,:] @ Wih_r.T + (b_ih+b_hh)   (big matmul)
      phase 2: 128 sequential LSTM cell steps (h.T is the matmul stationary)
      phase 3: out = h_cap @ Wout.T + bout
  - Ragged lengths: the recurrence runs unmasked; h is *captured* into Hf at
    t == len[b]-1 via a per-partition one-hot scalar (off the critical path).
    This is exact: for t >= len the reference state is frozen, so the captured
    h_{len-1} equals the reference h_T.

Gate order is permuted (i,f,g,o) -> (i,f,o,g) on the host so sigmoid applies
to one contiguous [.,900] slab and tanh to [.,300].
"""

import numpy as np
import ml_dtypes

import concourse.bass as bass
import concourse.tile as tile
from concourse import mybir
from concourse.bass_utils import run_bass_kernel_spmd

BF16 = ml_dtypes.bfloat16

# Model dims (hardcoded per spec nn_ABSA_Lstm_377957122440)
VOCAB, TVOCAB, D, H, C, B, T = 100000, 2000, 300, 300, 3, 256, 128
NCORES = 8
BL = B // NCORES          # 32 local batch
KT = 3                    # K tiles of 128 covering D(+1 bias row) / H
TC = T // 4               # 32 M-tiles of (4 t's x 32 b) in phase 1
NCH = 4                   # 1200 gate dims as 4 gate-major chunks of 300
CH = 300

_cache = {}


def _build_graph(legalize=True, debug=False, t_steps=T, reps=1,
                 trace_sim=False, variant=4):
    nc = bass.Bass()
    f32 = mybir.dt.float32
    bf16 = mybir.dt.bfloat16

    # ---- DRAM I/O ----
    xT = nc.dram_tensor("xT", [128, TC, KT, 128], bf16, kind="ExternalInput")
    wihT = nc.dram_tensor("wihT", [128, KT, 1200], bf16, kind="ExternalInput")
    whhT = nc.dram_tensor("whhT", [128, KT, 1200], bf16, kind="ExternalInput")
    wot = nc.dram_tensor("wot", [128, KT, 8], bf16, kind="ExternalInput")
    boutb = nc.dram_tensor("boutb", [BL, C], f32, kind="ExternalInput")
    h0T = nc.dram_tensor("h0T", [128, KT, BL], bf16, kind="ExternalInput")
    c0 = nc.dram_tensor("c0", [BL, H], f32, kind="ExternalInput")
    mlast = nc.dram_tensor("mlast", [BL, T], f32, kind="ExternalInput")
    ident = nc.dram_tensor("ident", [128, 32], bf16, kind="ExternalInput")
    out = nc.dram_tensor("out", [BL, C], f32, kind="ExternalOutput")
    dbg = None
    if debug:
        dbg = {
            "dbg_gx": nc.dram_tensor("dbg_gx", [128, TC, 1200], bf16,
                                     kind="ExternalOutput"),
            "dbg_gates": nc.dram_tensor("dbg_gates", [BL, 1200], f32,
                                        kind="ExternalOutput"),
            "dbg_h": nc.dram_tensor("dbg_h", [BL, H], bf16,
                                    kind="ExternalOutput"),
            "dbg_hT": nc.dram_tensor("dbg_hT", [128, KT, BL], bf16,
                                     kind="ExternalOutput"),
        }

    with tile.TileContext(nc, trace_sim=trace_sim) as tc:
        for _ in range(reps):
            _body(nc, tc, xT, wihT, whhT, wot, boutb, h0T, c0, mlast, ident,
                  out, dbg, t_steps, variant)
    if legalize:
        _legalize_single_wait(nc)
    return nc


def _legalize_single_wait(nc):
    """This walrus build accepts at most ONE sync wait per instruction.
    Hoist extra waits emitted by Tile onto standalone EventSemaphore
    instructions placed immediately before the offender on the same engine."""
    for fn in nc.m.functions:
        for b in fn.blocks:
            out = []
            for inst in b.instructions:
                si = getattr(inst, "sync_info", None)
                if si is not None and si.on_wait and len(si.on_wait) > 1:
                    for w in si.on_wait[:-1]:
                        out.append(mybir.InstEventSemaphore(
                            name=nc.get_next_instruction_name(),
                            engine=inst.engine,
                            ins=[], outs=[],
                            sync_info=mybir.SyncInfo(on_wait=[w], on_update=[]),
                        ))
                    si.on_wait = [si.on_wait[-1]]
                out.append(inst)
            b.instructions[:] = out


def TileCtx(nc):
    return tile.TileContext(nc)


def _body(nc, tc, xT, wihT, whhT, wot, boutb, h0T, c0, mlast, ident, out, dbg=None, t_steps=T, variant=4):
    f32 = mybir.dt.float32
    bf16 = mybir.dt.bfloat16
    Sig = mybir.ActivationFunctionType.Sigmoid
    Tanh = mybir.ActivationFunctionType.Tanh
    MUL = mybir.AluOpType.mult
    ADD = mybir.AluOpType.add

    from contextlib import ExitStack

    with ExitStack() as ctx:
        singles = ctx.enter_context(tc.tile_pool(name="singles", bufs=1))

        # ---- resident tiles ----
        whh_sb = singles.tile([128, KT, 1200], bf16)
        nc.sync.dma_start(out=whh_sb, in_=whhT[:, :, :])
        mlast_sb = singles.tile([BL, T], f32)
        nc.sync.dma_start(out=mlast_sb, in_=mlast[:, :])
        ident_sb = singles.tile([128, 32], bf16)
        nc.sync.dma_start(out=ident_sb, in_=ident[:, :])
        boutb_sb = singles.tile([BL, C], f32)
        nc.sync.dma_start(out=boutb_sb, in_=boutb[:, :])

        gx_all = singles.tile([128, TC, 1200], bf16)
        xT_sb = singles.tile([128, TC, KT, 128], bf16)
        nc.sync.dma_start(out=xT_sb, in_=xT[:, :, :, :])

        c_sb = singles.tile([BL, H], bf16)
        nc.gpsimd.dma_start(out=c_sb, in_=c0[:, :])
        Hf = singles.tile([BL, H], f32)
        nc.vector.memset(Hf, 0.0)
        # ping-pong transposed hidden state [kt, 128, 32]
        hT = [singles.tile([128, KT, BL], bf16, name=f"hT{i}", tag=f"hT{i}")
              for i in range(2)]
        nc.vector.memset(hT[1], 0.0)
        nc.sync.dma_start(out=hT[0], in_=h0T[:, :, :])

        # ===== phases 1+2 fused: gates_x tiles computed inside the loop =====
        # phase-1 M-tile tc is produced in two 2-gate bursts at steps where
        # t%4==0 / t%4==2, LAG tiles ahead of consumption.
        LAG = 2
        wih_sb = singles.tile([128, KT, 1200], bf16)
        nc.sync.dma_start(out=wih_sb, in_=wihT[:, :, :])

        with ExitStack() as p2:
            pgpool = p2.enter_context(tc.tile_pool(name="pgp", bufs=1, space="PSUM"))
            ptpool = p2.enter_context(tc.tile_pool(name="ptp", bufs=2, space="PSUM"))
            pspool = p2.enter_context(tc.tile_pool(name="psp", bufs=1, space="PSUM"))
            ew = p2.enter_context(tc.tile_pool(name="ew", bufs=2))

            def p1_burst(tcix, half):
                # compute gx_all[:, tcix, half*600 : half*600+600]
                ps = pspool.tile([128, 2, 512], f32, name=f"ps{tcix}_{half}",
                                 tag="ps")
                for k in range(KT):
                    for jj in range(2):
                        j = 2 * half + jj
                        nc.tensor.matmul(
                            ps[:, jj, 0:CH],
                            lhsT=xT_sb[:, tcix, k, :],
                            rhs=wih_sb[:, k, j * CH:(j + 1) * CH],
                            start=(k == 0), stop=(k == KT - 1),
                        )
                for jj in range(2):
                    j = 2 * half + jj
                    nc.any.tensor_copy(
                        gx_all[:, tcix, j * CH:(j + 1) * CH], ps[:, jj, 0:CH])

            nprod = (t_steps + 3) // 4 if t_steps else 0
            for tcix in range(min(LAG, nprod)):
                p1_burst(tcix, 0)
                p1_burst(tcix, 1)

            for t in range(t_steps):
                tcix, tt = t // 4, t % 4
                cur, nxt = hT[t % 2], hT[(t + 1) % 2]

                # gates PSUM, banks in gate order [g | i f | o] as three
                # tiles so per-bank dependency release lets each activation
                # start as soon as its own bank's matmuls finish. gx is
                # injected FIRST via identity stationary (no dependency on h
                # -> overlaps the previous step's elementwise chain).
                pg_g = pgpool.tile([BL, 512], f32, name=f"pg_g{t}", tag="pg_g")
                pg_if = pgpool.tile([BL, 2, 512], f32, name=f"pg_if{t}",
                                    tag="pg_if")
                pg_o = pgpool.tile([BL, 512], f32, name=f"pg_o{t}", tag="pg_o")
                banks = [pg_g[:, 0:CH], pg_if[:, 0, 0:CH], pg_if[:, 1, 0:CH],
                         pg_o[:, 0:CH]]
                for j in range(NCH):
                    nc.tensor.matmul(
                        banks[j],
                        lhsT=ident_sb[32 * tt:32 * tt + 32, :],
                        rhs=gx_all[32 * tt:32 * tt + 32, tcix,
                                   j * CH:(j + 1) * CH],
                        start=True, stop=False,
                        tile_position=(32 * tt, 0),
                    )
                for j in range(NCH):
                    for k in range(KT):
                        nc.tensor.matmul(
                            banks[j],
                            lhsT=cur[:, k, :],
                            rhs=whh_sb[:, k, j * CH:(j + 1) * CH],
                            start=False, stop=(k == KT - 1),
                        )

                if variant < 2:
                    continue
                if dbg is not None and t == 0:
                    gdbg = ew.tile([BL, NCH, CH], f32)
                    for j in range(NCH):
                        nc.vector.tensor_copy(gdbg[:, j, :], banks[j])
                    nc.sync.dma_start(out=dbg["dbg_gates"][:, :],
                                      in_=gdbg.rearrange("p n c -> p (n c)"))

                # activations straight from PSUM (bf16 out -> DVE 2x mode):
                # tanh(g) as soon as bank g lands, combined sigmoid(i,f),
                # sigmoid(o) last (only needed after tanh(c)).
                gg = ew.tile([BL, H], bf16)
                nc.scalar.activation(gg, pg_g[:, 0:CH], Tanh)
                sig = ew.tile([BL, 900], bf16)
                nc.scalar.activation(
                    sig[:, 0:600].rearrange("p (n c) -> p n c", n=2),
                    pg_if[:, :, 0:CH], Sig)
                nc.scalar.activation(sig[:, 600:900], pg_o[:, 0:CH], Sig)

                if variant < 3:
                    continue
                # c = f*c + i*g   (bf16 tensor_tensor -> 2x mode)
                t_ig = ew.tile([BL, H], bf16)
                nc.gpsimd.tensor_mul(t_ig, sig[:, 0:H], gg)
                t_fc = ew.tile([BL, H], bf16)
                nc.vector.tensor_mul(t_fc, sig[:, H:2 * H], c_sb)
                nc.vector.tensor_add(c_sb, t_fc, t_ig)

                # h = o * tanh(c), split so chunk a (h-dims 0:128) finishes
                # first and unblocks the k=0 transpose/copy/matmuls early
                th = ew.tile([BL, H], bf16)
                h_bf = ew.tile([BL, H], bf16)
                nc.scalar.activation(th[:, 0:128], c_sb[:, 0:128], Tanh)
                nc.vector.tensor_mul(h_bf[:, 0:128], sig[:, 600:728],
                                     th[:, 0:128])
                nc.scalar.activation(th[:, 128:H], c_sb[:, 128:H], Tanh)
                nc.vector.tensor_mul(h_bf[:, 128:H], sig[:, 728:900],
                                     th[:, 128:H])

                if dbg is not None and t == 0:
                    nc.sync.dma_start(out=dbg["dbg_h"][:, :], in_=h_bf)

                # capture h at t == len-1 (gpsimd, off critical path):
                # Hf += (h - Hf) * mlast[:, t]
                dh = ew.tile([BL, H], f32)
                nc.gpsimd.tensor_sub(dh, h_bf, Hf)
                nc.vector.scalar_tensor_tensor(
                    Hf, dh, mlast_sb[:, t:t + 1], Hf, MUL, ADD)

                if variant < 4:
                    continue
                # transpose h for next step's stationary
                for k in range(KT):
                    w = 128 if k < 2 else H - 256
                    pt = ptpool.tile([128, BL], bf16)
                    nc.tensor.transpose(
                        pt[0:w, :], h_bf[:, 128 * k:128 * k + w],
                        ident_sb[0:32, :])
                    nc.any.tensor_copy(nxt[0:w, k, :], pt[0:w, :])
                if dbg is not None and t == 0:
                    nc.sync.dma_start(out=dbg["dbg_hT"][:, :, :], in_=nxt)

                # phase-1 filler burst for a future tile (keeps PE warm)
                if tt == 0 and tcix + LAG < nprod:
                    p1_burst(tcix + LAG, 0)
                elif tt == 2 and tcix + LAG < nprod:
                    p1_burst(tcix + LAG, 1)

            # leftover phase-1 tiles when t_steps < T (debug builds)
            for tcix in range(min(LAG, nprod), nprod):
                pass

        if dbg is not None:
            nc.sync.dma_start(out=dbg["dbg_gx"][:, :, :], in_=gx_all)

        # ================= phase 3: out = Hf @ WoutT + bout =================
        with ExitStack() as p3:
            fp = p3.enter_context(tc.tile_pool(name="fp", bufs=1))
            fps = p3.enter_context(tc.tile_pool(name="fps", bufs=4, space="PSUM"))
            wot_sb = fp.tile([128, KT, 8], bf16)
            nc.sync.dma_start(out=wot_sb, in_=wot[:, :, :])
            hf_bf = fp.tile([BL, H], bf16)
            nc.vector.tensor_copy(hf_bf, Hf)
            hfT = fp.tile([128, KT, BL], bf16)
            nc.vector.memset(hfT, 0.0)
            for k in range(KT):
                w = 128 if k < 2 else H - 256
                pt = fps.tile([128, BL], bf16)
                nc.tensor.transpose(pt[0:w, :], hf_bf[:, 128 * k:128 * k + w],
                                    ident_sb[0:32, :])
                nc.any.tensor_copy(hfT[0:w, k, :], pt[0:w, :])
            po = fps.tile([BL, 8], f32)
            for k in range(KT):
                nc.tensor.matmul(po[:, 0:C], lhsT=hfT[:, k, :], rhs=wot_sb[:, k, 0:C],
                                 start=(k == 0), stop=(k == KT - 1))
            o_sb = fp.tile([BL, C], f32)
            nc.vector.tensor_add(o_sb, po[:, 0:C], boutb_sb)
            nc.sync.dma_start(out=out[:, :], in_=o_sb)


def _prep_inputs(sent, target, lens, emb, Wih, Whh, b_ih, b_hh, h0, c0,
                 Wout, bout):
    """Host-side shard + layout packing (data movement / tiny reindexing only)."""
    # permute gate order (i,f,g,o) -> (g,i,f,o)
    perm = np.concatenate([np.arange(600, 900), np.arange(0, 300),
                           np.arange(300, 600), np.arange(900, 1200)])
    wih_r = Wih[perm].astype(np.float32)          # [1200, 300]
    whh_r = Whh[perm].astype(np.float32)
    bias_r = (b_ih + b_hh)[perm].astype(np.float32)

    # [p, kt, n] with row D==bias, zero padded
    wihT = np.zeros((128, KT, 1200), np.float32)
    whhT = np.zeros((128, KT, 1200), np.float32)
    for k in range(KT):
        lo, hi = 128 * k, min(128 * (k + 1), D)
        wihT[0:hi - lo, k, :] = wih_r[:, lo:hi].T
        whhT[0:hi - lo, k, :] = whh_r[:, lo:hi].T
    wihT[D - 256, 2, :] = bias_r                   # ones-row partner
    wot = np.zeros((128, KT, 8), np.float32)
    for k in range(KT):
        lo, hi = 128 * k, min(128 * (k + 1), H)
        wot[0:hi - lo, k, 0:C] = Wout[:, lo:hi].T

    ident = np.tile(np.eye(32, dtype=np.float32), (4, 1))

    in_maps = []
    for ci in range(NCORES):
        sl = slice(ci * BL, (ci + 1) * BL)
        x = emb[sent[sl]].astype(np.float32)       # [32, 128, 300] gather
        xT = np.zeros((128, TC, KT, 128), np.float32)
        # lhsT layout: xT[p=dk, tc, kt, m=32tt+b] = x[b, 4tc+tt, 128kt+dk]
        xr = x.transpose(1, 0, 2).reshape(TC, 4, BL, D)   # [tc, tt, b, d]
        xr = xr.reshape(TC, 128, D)                       # [tc, m, d]
        for k in range(KT):
            lo, hi = 128 * k, min(128 * (k + 1), D)
            xT[0:hi - lo, :, k, :] = xr[:, :, lo:hi].transpose(2, 0, 1)
        xT[D - 256, :, 2, :] = 1.0                 # bias ones-row

        h0T = np.zeros((128, KT, BL), np.float32)
        for k in range(KT):
            lo, hi = 128 * k, min(128 * (k + 1), H)
            h0T[0:hi - lo, k, :] = h0[sl, lo:hi].T

        lloc = lens[sl].astype(np.int64)
        mlast = np.zeros((BL, T), np.float32)
        mlast[np.arange(BL), np.clip(lloc - 1, 0, T - 1)] = 1.0

        in_maps.append({
            "xT": xT.astype(BF16),
            "wihT": wihT.astype(BF16),
            "whhT": whhT.astype(BF16),
            "wot": wot.astype(BF16),
            "boutb": np.tile(bout.astype(np.float32), (BL, 1)),
            "h0T": h0T.astype(BF16),
            "c0": c0[sl].astype(np.float32),
            "mlast": mlast,
            "ident": ident.astype(BF16),
        })
    return in_maps


def kernel(**inputs):
    if "nc" not in _cache:
        _cache["nc"] = _build_graph()
    nc = _cache["nc"]
    in_maps = _prep_inputs(**inputs)
    res = run_bass_kernel_spmd(nc, in_maps, core_ids=list(range(NCORES)))
    outs = [res.results[i]["out"] for i in range(NCORES)]
    return np.concatenate(outs, axis=0).astype(np.float32)
